# revision 7
# baseline (speedup 1.0000x reference)
"""Trainium2 Bass kernel for CausalTrilinearBCNAttention.

Math (reference, per batch b, with counts[t] = t+1):
    Q = x @ Wq.T ; K = x @ Wk.T ; Z = cumsum(x)/counts
    a_b = Q @ V_b ; c_b = cumsum(K @ W_b) ; bil = (a_b*c_b) @ U_b.T
    a_t = Q @ V_t ; c_t = cumsum(K @ W_t) ; z_t = Z @ X_t
    tri = (a_t*z_t*c_t) @ U_t.T
    out = ((bil + alpha*tri)/counts + bias_b + alpha*bias_t) @ Wo.T

Everything is linear around the cumsums, so the big projections fold into
small [D,R] matrices on the host:
    a_b  = x @ (Wq.T@V_b)        kw_b = x @ (Wk.T@W_b)
    a_t  = x @ (Wq.T@V_t)        kw_t = x @ (Wk.T@W_t)
    zx   = x @ X_t
    c_b  = cumsum(kw_b), c_t = cumsum(kw_t), zcum = cumsum(zx)
    u_b  = a_b * c_b
    u_t  = a_t * zcum * c_t * (1/counts)          (one 1/counts here)
    out  = (u_b @ (Wo@U_b).T + u_t @ (alpha*Wo@U_t).T) * (1/counts)
           + (bias_b + alpha*bias_t) @ Wo.T       (bias term added on host)

Sharding: 8 cores = (B=4) x (two T-halves of 1024). The second-half cores
receive host-computed cumsum prefix offsets (sum of the first half of x,
pushed through the same small matrices) as the scan initial values.

On-chip layout: mid tensors live as [R(=256, 2 partition chunks), T] with T
on the free dim, so the causal cumsums are native `tensor_tensor_scan`
instructions and the final matmul consumes u directly as lhsT.
Matmul operands are float32r (full PE rate for N>=256); the host pre-rounds
them to the f32r grid (11-bit mantissa, RNE) which hardware matches exactly.
"""

import numpy as np

D = 1024
R = 256
B = 4
T = 2048
TL = 1024            # local T per core
NCORES = 8
NRC = 10             # 5*R/128 column chunks of P_all
NTB = TL // 128      # T blocks of 128
W_P = 5 * R          # 1280 columns of P_all
W_PX = W_P + TL      # P_all | xT combined width

_CACHE = {}
LAST_RESULTS = None  # BassKernelResults of the most recent run (for test.py)


def round_f32r(a):
    """Round fp32 to the float32r grid (11-bit mantissa, RNE) — matches HW."""
    b = np.ascontiguousarray(a, np.float32).view(np.uint32)
    rb = (b >> 12) & 1
    return ((b + 0x7FF + rb) & 0xFFFFF000).astype(np.uint32).view(np.float32)


def _build_nc():
    import concourse.bacc as bacc
    import concourse.bass as bass
    import concourse.tile as tile
    import concourse.mybir as mybir

    f32 = mybir.dt.float32
    f32r = mybir.dt.float32r
    Copy = mybir.ActivationFunctionType.Copy
    Alu = mybir.AluOpType

    nc = bacc.Bacc()
    # PX = [P_all | xT] along free dim: one DMA per 128-row D-chunk
    PX_d = nc.dram_tensor("PX", [D, W_PX], f32r, kind="ExternalInput")
    G_d = nc.dram_tensor("G_all", [R, 2 * D], f32r, kind="ExternalInput")
    prev_d = nc.dram_tensor("prev", [128, 6], f32, kind="ExternalInput")
    invp_d = nc.dram_tensor("invc_p", [128, NTB], f32, kind="ExternalInput")
    invr_d = nc.dram_tensor("invc_row", [TL], f32, kind="ExternalInput")
    out_d = nc.dram_tensor("out", [TL, D], f32, kind="ExternalOutput")

    PX_r = PX_d.ap().rearrange("(c p) f -> p c f", p=128)   # [128, 8, 2304]
    G_r = G_d.ap().rearrange("(c p) f -> p c f", p=128)     # [128, 2, 2048]

    mm = nc.tensor.matmul

    with tile.TileContext(nc) as tc:
        with (
            tc.tile_pool(name="persist", bufs=1) as persist,
            tc.tile_pool(name="outp", bufs=3) as outp,
            tc.tile_pool(name="pa", bufs=5, space="PSUM") as pa,
            tc.tile_pool(name="pf", bufs=3, space="PSUM") as pf,
        ):
            PX = persist.tile([128, 8, W_PX], f32r, name="PX_sb")
            G = persist.tile([128, 2, 2 * D], f32r, name="G_sb")
            prev = persist.tile([128, 6], f32, name="prev_sb")
            invp = persist.tile([128, NTB], f32, name="invp_sb")
            invb = persist.tile([128, TL], f32, name="invb_sb")
            # A chunks: 0-1 kw_b, 2-3 kw_t, 4-5 zx, 6-7 a_b, 8-9 a_t
            A = [persist.tile([128, 2, TL], f32, name=f"A{i}_sb") for i in range(5)]
            # C chunks: 0-1 c_b, 2-3 c_t, 4-5 zcum
            C = [persist.tile([128, 2, TL], f32, name=f"C{i}_sb") for i in range(3)]
            # U: 0 u_b, 1 u_t (each [128, 2(R-chunks), TL]), f32r for the PE
            U = [persist.tile([128, 2, TL], f32r, name=f"U{i}_sb") for i in range(2)]

            # ---- input DMAs ----
            nc.sync.dma_start(G[:, :, :], G_r)
            nc.sync.dma_start(prev[:, :], prev_d.ap())
            nc.sync.dma_start(invp[:, :], invp_d.ap())
            invr_ap = invr_d.ap()
            invr_bcast = bass.AP(
                tensor=invr_ap.tensor,
                offset=invr_ap.offset,
                ap=[[0, 128]] + [list(a) for a in invr_ap.ap],
            )
            nc.gpsimd.dma_start(out=invb[:, :], in_=invr_bcast)
            for d in range(8):
                nc.sync.dma_start(PX[:, d, :], PX_r[:, d, :])

            # ---- stage A: [kw_b kw_t zx a_b a_t]^T = P_all^T x^T ----
            # column-chunk rc of P_all -> A[rc//2][:, rc%2, :]
            for rc in range(NRC):
                for tcc in range(TL // 512):
                    ps = pa.tile([128, 512], f32, name="psA", tag="psA")
                    for d in range(8):
                        mm(
                            ps,
                            PX[:, d, rc * 128:(rc + 1) * 128],
                            PX[:, d, W_P + tcc * 512:W_P + (tcc + 1) * 512],
                            start=(d == 0),
                            stop=(d == 7),
                        )
                    nc.scalar.copy(A[rc // 2][:, rc % 2, tcc * 512:(tcc + 1) * 512], ps)
                # causal cumsums as soon as their producer chunks are done
                if rc in (1, 3, 5):
                    j = rc // 2
                    for h in range(2):
                        nc.vector.tensor_tensor_scan(
                            C[j][:, h, :],
                            A[j][:, h, :],
                            A[j][:, h, :],
                            initial=prev[:, 2 * j + h:2 * j + h + 1],
                            op0=Alu.add,
                            op1=Alu.bypass,
                        )

            # ---- u products ----
            for h in range(2):
                nc.gpsimd.tensor_mul(U[0][:, h, :], A[3][:, h, :], C[0][:, h, :])
                nc.vector.tensor_mul(U[1][:, h, :], A[4][:, h, :], C[2][:, h, :])
                nc.vector.tensor_mul(U[1][:, h, :], U[1][:, h, :], C[1][:, h, :])
                nc.gpsimd.tensor_mul(U[1][:, h, :], U[1][:, h, :], invb[:, :])

            # ---- final: out[t, d] = sum_r u[r, t] G[r, d], scaled by 1/counts ----
            for tb in range(NTB):
                ot = outp.tile([128, D], f32, name="out_t", tag="out_t")
                tsl = slice(tb * 128, (tb + 1) * 128)
                for nh in range(2):
                    ps = pf.tile([128, 512], f32, name="psF", tag="psF")
                    dsl = slice(nh * 512, (nh + 1) * 512)
                    dsl_t = slice(D + nh * 512, D + (nh + 1) * 512)
                    mm(ps, U[0][:, 0, tsl], G[:, 0, dsl], start=True, stop=False)
                    mm(ps, U[0][:, 1, tsl], G[:, 1, dsl], start=False, stop=False)
                    mm(ps, U[1][:, 0, tsl], G[:, 0, dsl_t], start=False, stop=False)
                    mm(ps, U[1][:, 1, tsl], G[:, 1, dsl_t], start=False, stop=True)
                    nc.scalar.activation(
                        ot[:, dsl], ps, Copy, scale=invp[:, tb:tb + 1]
                    )
                nc.sync.dma_start(out_d.ap()[tsl, :], ot[:, :])
    nc.compile()
    return nc


def get_nc():
    if "nc" not in _CACHE:
        _CACHE["nc"] = _build_nc()
    return _CACHE["nc"]


def make_in_maps(inputs):
    """Host-side fusion + sharding. Returns (in_maps, bias_out)."""
    f = lambda k: np.ascontiguousarray(np.asarray(inputs[k], dtype=np.float32))
    x = f("x")
    Wq, Wk, Wo = f("Wq"), f("Wk"), f("Wo")
    U_b, V_b, W_b = f("U_b"), f("V_b"), f("W_b")
    U_t, V_t, W_t, X_t = f("U_t"), f("V_t"), f("W_t"), f("X_t")
    bias_b, bias_t = f("bias_b"), f("bias_t")
    alpha = float(np.asarray(inputs["alpha"]))

    P_cb = Wk.T @ W_b
    P_ct = Wk.T @ W_t
    P_ab = Wq.T @ V_b
    P_at = Wq.T @ V_t
    P_all = round_f32r(np.concatenate([P_cb, P_ct, X_t, P_ab, P_at], axis=1))
    Gb = (Wo @ U_b).T
    Gt = alpha * (Wo @ U_t).T
    G_all = round_f32r(np.concatenate([Gb, Gt], axis=1))

    xr = round_f32r(x)  # device consumes the f32r-rounded x
    xs = xr[:, :TL, :].astype(np.float64).sum(axis=1).astype(np.float32)  # [B, D]
    prev_cb = xs @ P_all[:, 0:R]
    prev_ct = xs @ P_all[:, R:2 * R]
    prev_z = xs @ P_all[:, 2 * R:3 * R]

    in_maps = []
    for core in range(NCORES):
        b, h = divmod(core, 2)
        xT = np.ascontiguousarray(xr[b, h * TL:(h + 1) * TL, :].T)
        PX = np.ascontiguousarray(np.concatenate([P_all, xT], axis=1))
        if h == 0:
            prev = np.zeros((128, 6), np.float32)
        else:
            prev = np.ascontiguousarray(
                np.stack(
                    [
                        prev_cb[b, :128], prev_cb[b, 128:],
                        prev_ct[b, :128], prev_ct[b, 128:],
                        prev_z[b, :128], prev_z[b, 128:],
                    ],
                    axis=1,
                )
            )
        counts = np.arange(h * TL + 1, (h + 1) * TL + 1, dtype=np.float64)
        invc = (1.0 / counts).astype(np.float32)
        invp = np.ascontiguousarray(invc.reshape(NTB, 128).T)
        in_maps.append(
            {
                "PX": PX,
                "G_all": G_all,
                "prev": prev,
                "invc_p": invp,
                "invc_row": np.ascontiguousarray(invc),
            }
        )
    bias_out = (bias_b + alpha * bias_t) @ Wo.T
    return in_maps, bias_out


def kernel(**inputs):
    global LAST_RESULTS
    from concourse.bass_utils import run_bass_kernel_spmd

    in_maps, bias_out = make_in_maps(inputs)
    nc = get_nc()
    res = run_bass_kernel_spmd(nc, in_maps, core_ids=list(range(NCORES)))
    LAST_RESULTS = res
    out = np.empty((B, T, D), np.float32)
    for core in range(NCORES):
        b, h = divmod(core, 2)
        out[b, h * TL:(h + 1) * TL, :] = res.results[core]["out"]
    if np.any(bias_out != 0.0):
        out += bias_out[None, None, :]
    return out


# revision 8
# speedup vs baseline: 1.2870x; 1.2870x over previous
"""Trainium2 Bass kernel for CausalTrilinearBCNAttention.

Math (reference, per batch b, with counts[t] = t+1):
    Q = x @ Wq.T ; K = x @ Wk.T ; Z = cumsum(x)/counts
    a_b = Q @ V_b ; c_b = cumsum(K @ W_b) ; bil = (a_b*c_b) @ U_b.T
    a_t = Q @ V_t ; c_t = cumsum(K @ W_t) ; z_t = Z @ X_t
    tri = (a_t*z_t*c_t) @ U_t.T
    out = ((bil + alpha*tri)/counts + bias_b + alpha*bias_t) @ Wo.T

Everything is linear around the cumsums, so the big projections fold into
small [D,R] matrices on the host:
    a_b  = x @ (Wq.T@V_b)        kw_b = x @ (Wk.T@W_b)
    a_t  = x @ (Wq.T@V_t)        kw_t = x @ (Wk.T@W_t)
    zx   = x @ X_t
    c_b  = cumsum(kw_b), c_t = cumsum(kw_t), zcum = cumsum(zx)
    u_b  = a_b * c_b
    u_t  = a_t * zcum * (c_t / counts)            (one 1/counts here)
    out  = (u_b @ (Wo@U_b).T + u_t @ (alpha*Wo@U_t).T) * (1/counts)
           + (bias_b + alpha*bias_t) @ Wo.T       (bias term added on host)

Sharding: 8 cores = (B=4) x (two T-halves of 1024). The second-half cores
receive host-computed cumsum prefix offsets (sum of the first half of x,
pushed through the same small matrices) as the scan initial values.

On-chip layout: mid tensors live as [R(=256, 2 partition chunks), T] with T
on the free dim, so the causal cumsums are native `tensor_tensor_scan`
instructions and the final matmul consumes u directly as lhsT.

dtypes: stage-A operands (x and the folded P matrices) are bf16 (halves the
input-DMA prologue); cumsums/elementwise run in fp32; the final matmul runs
in float32r (fp32 with 11-bit mantissa, full PE rate) to keep the last
projection accurate. A PE warmup block of dummy matmuls spans the input-DMA
wait so the HAM clock gate is at 2.4 GHz when real work starts.
"""

import numpy as np

D = 1024
R = 256
B = 4
T = 2048
TL = 1024            # local T per core
NCORES = 8
NRC = 10             # 5*R/128 column chunks of P_all
NTB = TL // 128      # T blocks of 128
W_P = 5 * R          # 1280 columns of P_all
W_PX = W_P + TL      # P_all | xT combined width
N_WARMUP = 120       # dummy PE matmuls spanning the input-DMA wait

_CACHE = {}
LAST_RESULTS = None  # BassKernelResults of the most recent run (for test.py)


def round_f32r(a):
    """Round fp32 to the float32r grid (11-bit mantissa, RNE) — matches HW."""
    b = np.ascontiguousarray(a, np.float32).view(np.uint32)
    rb = (b >> 12) & 1
    return ((b + 0x7FF + rb) & 0xFFFFF000).astype(np.uint32).view(np.float32)


def _build_nc():
    import concourse.bacc as bacc
    import concourse.bass as bass
    import concourse.tile as tile
    import concourse.mybir as mybir

    f32 = mybir.dt.float32
    f32r = mybir.dt.float32r
    bf16 = mybir.dt.bfloat16
    Copy = mybir.ActivationFunctionType.Copy
    Alu = mybir.AluOpType

    nc = bacc.Bacc()
    # PX = [P_all | xT] along free dim (bf16): one DMA per 128-row D-chunk.
    # P_all column order: [P_cb | P_ct | X_t | P_at | P_ab]
    PX_d = nc.dram_tensor("PX", [D, W_PX], bf16, kind="ExternalInput")
    G_d = nc.dram_tensor("G_all", [R, 2 * D], f32r, kind="ExternalInput")
    prev_d = nc.dram_tensor("prev", [128, 6], f32, kind="ExternalInput")
    invp_d = nc.dram_tensor("invc_p", [128, NTB], f32, kind="ExternalInput")
    invr_d = nc.dram_tensor("invc_row", [TL], f32, kind="ExternalInput")
    out_d = nc.dram_tensor("out", [TL, D], f32, kind="ExternalOutput")

    PX_r = PX_d.ap().rearrange("(c p) f -> p c f", p=128)   # [128, 8, 2304]
    G_r = G_d.ap().rearrange("(c p) f -> p c f", p=128)     # [128, 2, 2048]

    mm = nc.tensor.matmul

    with tile.TileContext(nc) as tc:
        with (
            tc.tile_pool(name="persist", bufs=1) as persist,
            tc.tile_pool(name="outp", bufs=3) as outp,
            tc.tile_pool(name="pa", bufs=5, space="PSUM") as pa,
            tc.tile_pool(name="pf", bufs=3, space="PSUM") as pf,
        ):
            PX = persist.tile([128, 8, W_PX], bf16, name="PX_sb")
            G = persist.tile([128, 2, 2 * D], f32r, name="G_sb")
            prev = persist.tile([128, 6], f32, name="prev_sb")
            invp = persist.tile([128, NTB], f32, name="invp_sb")
            invb = persist.tile([128, TL], f32, name="invb_sb")
            dummy = persist.tile([128, 128], bf16, name="dummy_sb")
            # A chunks: 0-1 kw_b, 2-3 kw_t, 4-5 zx, 6-7 a_t, 8-9 a_b
            A = [persist.tile([128, 2, TL], f32, name=f"A{i}_sb") for i in range(5)]
            # C chunks: 0-1 c_b, 2-3 c_t, 4-5 zcum
            C = [persist.tile([128, 2, TL], f32, name=f"C{i}_sb") for i in range(3)]
            # civ = c_t * (1/counts), computed during stage A
            CIV = persist.tile([128, 2, TL], f32, name="CIV_sb")
            # U: 0 u_b, 1 u_t (each [128, 2(R-chunks), TL]), f32r for the PE
            U = [persist.tile([128, 2, TL], f32r, name=f"U{i}_sb") for i in range(2)]

            # ---- PE warmup: keep the HAM clock gate open during input DMA ----
            nc.vector.memset(dummy[:, :], 0.5)
            for w in range(N_WARMUP):
                pw = pf.tile([128, 128], f32, name="psW", tag="psF")
                mm(pw, dummy[:, :], dummy[:, :], start=True, stop=True)

            # ---- input DMAs (PX first; G last — only needed by final stage) ----
            nc.sync.dma_start(prev[:, :], prev_d.ap())
            nc.sync.dma_start(invp[:, :], invp_d.ap())
            for d in range(8):
                nc.sync.dma_start(PX[:, d, :], PX_r[:, d, :])
            nc.sync.dma_start(G[:, :, :], G_r)
            invr_ap = invr_d.ap()
            invr_bcast = bass.AP(
                tensor=invr_ap.tensor,
                offset=invr_ap.offset,
                ap=[[0, 128]] + [list(a) for a in invr_ap.ap],
            )
            nc.gpsimd.dma_start(out=invb[:, :], in_=invr_bcast)

            # ---- stage A: [kw_b kw_t zx a_t a_b]^T = P_all^T x^T ----
            # column-chunk rc of P_all -> A[rc//2][:, rc%2, :]
            for rc in range(NRC):
                for tcc in range(TL // 512):
                    ps = pa.tile([128, 512], f32, name="psA", tag="psA")
                    for d in range(8):
                        mm(
                            ps,
                            PX[:, d, rc * 128:(rc + 1) * 128],
                            PX[:, d, W_P + tcc * 512:W_P + (tcc + 1) * 512],
                            start=(d == 0),
                            stop=(d == 7),
                        )
                    nc.scalar.copy(A[rc // 2][:, rc % 2, tcc * 512:(tcc + 1) * 512], ps)
                # causal cumsums as soon as their producer chunks are done,
                # chunked along T so downstream can start at the half mark
                if rc in (1, 3, 5):
                    j = rc // 2
                    for h in range(2):
                        nc.vector.tensor_tensor_scan(
                            C[j][:, h, 0:512],
                            A[j][:, h, 0:512],
                            A[j][:, h, 0:512],
                            initial=prev[:, 2 * j + h:2 * j + h + 1],
                            op0=Alu.add,
                            op1=Alu.bypass,
                        )
                        nc.vector.tensor_tensor_scan(
                            C[j][:, h, 512:TL],
                            A[j][:, h, 512:TL],
                            A[j][:, h, 512:TL],
                            initial=C[j][:, h, 511:512],
                            op0=Alu.add,
                            op1=Alu.bypass,
                        )
                    if j == 1:  # civ = c_t / counts, off the critical path
                        for h in range(2):
                            for tcc in range(2):
                                nc.gpsimd.tensor_mul(
                                    CIV[:, h, tcc * 512:(tcc + 1) * 512],
                                    C[1][:, h, tcc * 512:(tcc + 1) * 512],
                                    invb[:, tcc * 512:(tcc + 1) * 512],
                                )

            # ---- u products (chunked so final matmuls start at the half mark)
            # u_b = a_b * c_b ; u_t = (a_t * zcum) * civ
            for tcc in range(2):
                sl = slice(tcc * 512, (tcc + 1) * 512)
                for h in range(2):
                    nc.gpsimd.tensor_mul(U[0][:, h, sl], A[4][:, h, sl], C[0][:, h, sl])
                    nc.vector.tensor_mul(U[1][:, h, sl], A[3][:, h, sl], C[2][:, h, sl])
                    nc.vector.tensor_mul(U[1][:, h, sl], U[1][:, h, sl], CIV[:, h, sl])

            # ---- final: out[t, d] = sum_r u[r, t] G[r, d], scaled by 1/counts ----
            for tb in range(NTB):
                ot = outp.tile([128, D], f32, name="out_t", tag="out_t")
                tsl = slice(tb * 128, (tb + 1) * 128)
                for nh in range(2):
                    ps = pf.tile([128, 512], f32, name="psF", tag="psF")
                    dsl = slice(nh * 512, (nh + 1) * 512)
                    dsl_t = slice(D + nh * 512, D + (nh + 1) * 512)
                    mm(ps, U[0][:, 0, tsl], G[:, 0, dsl], start=True, stop=False)
                    mm(ps, U[0][:, 1, tsl], G[:, 1, dsl], start=False, stop=False)
                    mm(ps, U[1][:, 0, tsl], G[:, 0, dsl_t], start=False, stop=False)
                    mm(ps, U[1][:, 1, tsl], G[:, 1, dsl_t], start=False, stop=True)
                    nc.scalar.activation(
                        ot[:, dsl], ps, Copy, scale=invp[:, tb:tb + 1]
                    )
                nc.sync.dma_start(out_d.ap()[tsl, :], ot[:, :])
    nc.compile()
    return nc


def get_nc():
    if "nc" not in _CACHE:
        _CACHE["nc"] = _build_nc()
    return _CACHE["nc"]


def make_in_maps(inputs):
    """Host-side fusion + sharding. Returns (in_maps, bias_out)."""
    import ml_dtypes

    bf16 = ml_dtypes.bfloat16
    f = lambda k: np.ascontiguousarray(np.asarray(inputs[k], dtype=np.float32))
    x = f("x")
    Wq, Wk, Wo = f("Wq"), f("Wk"), f("Wo")
    U_b, V_b, W_b = f("U_b"), f("V_b"), f("W_b")
    U_t, V_t, W_t, X_t = f("U_t"), f("V_t"), f("W_t"), f("X_t")
    bias_b, bias_t = f("bias_b"), f("bias_t")
    alpha = float(np.asarray(inputs["alpha"]))

    P_cb = Wk.T @ W_b
    P_ct = Wk.T @ W_t
    P_ab = Wq.T @ V_b
    P_at = Wq.T @ V_t
    # column order: [P_cb | P_ct | X_t | P_at | P_ab]
    P_all = np.concatenate([P_cb, P_ct, X_t, P_at, P_ab], axis=1).astype(bf16)
    Gb = (Wo @ U_b).T
    Gt = alpha * (Wo @ U_t).T
    G_all = round_f32r(np.concatenate([Gb, Gt], axis=1))

    xb = x.astype(bf16)  # device consumes bf16 x
    P_f32 = P_all.astype(np.float32)
    xs = xb.astype(np.float64)[:, :TL, :].sum(axis=1).astype(np.float32)  # [B, D]
    prev_cb = xs @ P_f32[:, 0:R]
    prev_ct = xs @ P_f32[:, R:2 * R]
    prev_z = xs @ P_f32[:, 2 * R:3 * R]

    in_maps = []
    for core in range(NCORES):
        b, h = divmod(core, 2)
        xT = np.ascontiguousarray(xb[b, h * TL:(h + 1) * TL, :].T)
        PX = np.ascontiguousarray(np.concatenate([P_all, xT], axis=1))
        if h == 0:
            prev = np.zeros((128, 6), np.float32)
        else:
            prev = np.ascontiguousarray(
                np.stack(
                    [
                        prev_cb[b, :128], prev_cb[b, 128:],
                        prev_ct[b, :128], prev_ct[b, 128:],
                        prev_z[b, :128], prev_z[b, 128:],
                    ],
                    axis=1,
                )
            )
        counts = np.arange(h * TL + 1, (h + 1) * TL + 1, dtype=np.float64)
        invc = (1.0 / counts).astype(np.float32)
        invp = np.ascontiguousarray(invc.reshape(NTB, 128).T)
        in_maps.append(
            {
                "PX": PX,
                "G_all": G_all,
                "prev": prev,
                "invc_p": invp,
                "invc_row": np.ascontiguousarray(invc),
            }
        )
    bias_out = (bias_b + alpha * bias_t) @ Wo.T
    return in_maps, bias_out


def kernel(**inputs):
    global LAST_RESULTS
    from concourse.bass_utils import run_bass_kernel_spmd

    in_maps, bias_out = make_in_maps(inputs)
    nc = get_nc()
    res = run_bass_kernel_spmd(nc, in_maps, core_ids=list(range(NCORES)))
    LAST_RESULTS = res
    out = np.empty((B, T, D), np.float32)
    for core in range(NCORES):
        b, h = divmod(core, 2)
        out[b, h * TL:(h + 1) * TL, :] = res.results[core]["out"]
    if np.any(bias_out != 0.0):
        out += bias_out[None, None, :]
    return out


# revision 9
# speedup vs baseline: 1.3202x; 1.0258x over previous
"""Trainium2 Bass kernel for CausalTrilinearBCNAttention.

Math (reference, per batch b, with counts[t] = t+1):
    Q = x @ Wq.T ; K = x @ Wk.T ; Z = cumsum(x)/counts
    a_b = Q @ V_b ; c_b = cumsum(K @ W_b) ; bil = (a_b*c_b) @ U_b.T
    a_t = Q @ V_t ; c_t = cumsum(K @ W_t) ; z_t = Z @ X_t
    tri = (a_t*z_t*c_t) @ U_t.T
    out = ((bil + alpha*tri)/counts + bias_b + alpha*bias_t) @ Wo.T

Everything is linear around the cumsums, so the big projections fold into
small [D,R] matrices on the host:
    a_b  = x @ (Wq.T@V_b)        kw_b = x @ (Wk.T@W_b)
    a_t  = x @ (Wq.T@V_t)        kw_t = x @ (Wk.T@W_t)
    zx   = x @ X_t
    c_b  = cumsum(kw_b), c_t = cumsum(kw_t), zcum = cumsum(zx)
    u_b  = a_b * c_b
    u_t  = a_t * zcum * (c_t / counts)            (one 1/counts here)
    out  = (u_b @ (Wo@U_b).T + u_t @ (alpha*Wo@U_t).T) * (1/counts)
           + (bias_b + alpha*bias_t) @ Wo.T       (bias term added on host)

Sharding: 8 cores = (B=4) x (two T-halves of 1024). The second-half cores
receive host-computed cumsum prefix offsets (sum of the first half of x,
pushed through the same small matrices) as the scan initial values.

On-chip layout: mid tensors live as [R(=256, 2 partition chunks), T] with T
on the free dim, so the causal cumsums are native `tensor_tensor_scan`
instructions and the final matmul consumes u directly as lhsT.

dtypes: stage-A operands (x and the folded P matrices) are bf16 (halves the
input-DMA prologue); cumsums/elementwise run in fp32; the final matmul runs
in float32r (fp32 with 11-bit mantissa, full PE rate) to keep the last
projection accurate. A PE warmup block of dummy matmuls spans the input-DMA
wait so the HAM clock gate is at 2.4 GHz when real work starts.
"""

import numpy as np

D = 1024
R = 256
B = 4
T = 2048
TL = 1024            # local T per core
NCORES = 8
NRC = 10             # 5*R/128 column chunks of P_all
NTB = TL // 128      # T blocks of 128
W_P = 5 * R          # 1280 columns of P_all
W_PX = W_P + TL      # P_all | xT combined width
N_WARMUP = 80        # dummy PE matmuls spanning the input-DMA wait

_CACHE = {}
LAST_RESULTS = None  # BassKernelResults of the most recent run (for test.py)


def round_f32r(a):
    """Round fp32 to the float32r grid (11-bit mantissa, RNE) — matches HW."""
    b = np.ascontiguousarray(a, np.float32).view(np.uint32)
    rb = (b >> 12) & 1
    return ((b + 0x7FF + rb) & 0xFFFFF000).astype(np.uint32).view(np.float32)


def _build_nc():
    import concourse.bacc as bacc
    import concourse.bass as bass
    import concourse.tile as tile
    import concourse.mybir as mybir

    f32 = mybir.dt.float32
    f32r = mybir.dt.float32r
    bf16 = mybir.dt.bfloat16
    Copy = mybir.ActivationFunctionType.Copy
    Alu = mybir.AluOpType

    nc = bacc.Bacc()
    # PX = [P_all | xT] along free dim (bf16): one DMA per 128-row D-chunk.
    # P_all column order: [P_cb | P_ct | X_t | P_at | P_ab]
    PX_d = nc.dram_tensor("PX", [D, W_PX], bf16, kind="ExternalInput")
    G_d = nc.dram_tensor("G_all", [R, 2 * D], f32r, kind="ExternalInput")
    prev_d = nc.dram_tensor("prev", [128, 6], f32, kind="ExternalInput")
    invp_d = nc.dram_tensor("invc_p", [128, NTB], f32, kind="ExternalInput")
    invr_d = nc.dram_tensor("invc_row", [TL], f32, kind="ExternalInput")
    out_d = nc.dram_tensor("out", [TL, D], f32, kind="ExternalOutput")

    PX_r = PX_d.ap().rearrange("(c p) f -> p c f", p=128)   # [128, 8, 2304]
    G_r = G_d.ap().rearrange("(c p) f -> p c f", p=128)     # [128, 2, 2048]

    mm = nc.tensor.matmul

    with tile.TileContext(nc) as tc:
        with (
            tc.tile_pool(name="persist", bufs=1) as persist,
            tc.tile_pool(name="outp", bufs=3) as outp,
            tc.tile_pool(name="pa", bufs=5, space="PSUM") as pa,
            tc.tile_pool(name="pf", bufs=3, space="PSUM") as pf,
        ):
            PX = persist.tile([128, 8, W_PX], bf16, name="PX_sb")
            G = persist.tile([128, 2, 2 * D], f32r, name="G_sb")
            prev = persist.tile([128, 6], f32, name="prev_sb")
            invp = persist.tile([128, NTB], f32, name="invp_sb")
            invb = persist.tile([128, TL], f32, name="invb_sb")
            dummy = persist.tile([128, 128], bf16, name="dummy_sb")
            # A chunks: 0-1 kw_b, 2-3 kw_t, 4-5 zx, 6-7 a_t, 8-9 a_b
            A = [persist.tile([128, 2, TL], f32, name=f"A{i}_sb") for i in range(5)]
            # C chunks: 0-1 c_b, 2-3 c_t, 4-5 zcum
            C = [persist.tile([128, 2, TL], f32, name=f"C{i}_sb") for i in range(3)]
            # civ = c_t * (1/counts), computed during stage A
            CIV = persist.tile([128, 2, TL], f32, name="CIV_sb")
            # U: 0 u_b, 1 u_t (each [128, 2(R-chunks), TL]), f32r for the PE
            U = [persist.tile([128, 2, TL], f32r, name=f"U{i}_sb") for i in range(2)]

            # ---- PE warmup: keep the HAM clock gate open during input DMA ----
            nc.vector.memset(dummy[:, :], 0.5)
            for w in range(N_WARMUP):
                pw = pf.tile([128, 128], f32, name="psW", tag="psF")
                mm(pw, dummy[:, :], dummy[:, :], start=True, stop=True)

            # ---- input DMAs (PX first; G last — only needed by final stage) ----
            nc.sync.dma_start(prev[:, :], prev_d.ap())
            nc.sync.dma_start(invp[:, :], invp_d.ap())
            for d in range(8):
                nc.sync.dma_start(PX[:, d, :], PX_r[:, d, :])
            nc.sync.dma_start(G[:, :, :], G_r)
            invr_ap = invr_d.ap()
            invr_bcast = bass.AP(
                tensor=invr_ap.tensor,
                offset=invr_ap.offset,
                ap=[[0, 128]] + [list(a) for a in invr_ap.ap],
            )
            nc.gpsimd.dma_start(out=invb[:, :], in_=invr_bcast)

            # ---- stage A: [kw_b kw_t zx a_t a_b]^T = P_all^T x^T ----
            # column-chunk rc of P_all -> A[rc//2][:, rc%2, :]
            for rc in range(NRC):
                for tcc in range(TL // 512):
                    ps = pa.tile([128, 512], f32, name="psA", tag="psA")
                    for d in range(8):
                        mm(
                            ps,
                            PX[:, d, rc * 128:(rc + 1) * 128],
                            PX[:, d, W_P + tcc * 512:W_P + (tcc + 1) * 512],
                            start=(d == 0),
                            stop=(d == 7),
                        )
                    nc.scalar.copy(A[rc // 2][:, rc % 2, tcc * 512:(tcc + 1) * 512], ps)
                # causal cumsums as soon as their producer chunks are done,
                # chunked along T so downstream can start at the half mark
                if rc in (1, 3, 5):
                    j = rc // 2
                    for h in range(2):
                        nc.vector.tensor_tensor_scan(
                            C[j][:, h, 0:512],
                            A[j][:, h, 0:512],
                            A[j][:, h, 0:512],
                            initial=prev[:, 2 * j + h:2 * j + h + 1],
                            op0=Alu.add,
                            op1=Alu.bypass,
                        )
                        nc.vector.tensor_tensor_scan(
                            C[j][:, h, 512:TL],
                            A[j][:, h, 512:TL],
                            A[j][:, h, 512:TL],
                            initial=C[j][:, h, 511:512],
                            op0=Alu.add,
                            op1=Alu.bypass,
                        )
                    if j == 1:  # civ = c_t / counts, off the critical path
                        for h in range(2):
                            for tcc in range(2):
                                nc.gpsimd.tensor_mul(
                                    CIV[:, h, tcc * 512:(tcc + 1) * 512],
                                    C[1][:, h, tcc * 512:(tcc + 1) * 512],
                                    invb[:, tcc * 512:(tcc + 1) * 512],
                                )

            # ---- u products (chunked so final matmuls start at the half mark)
            # u_b = a_b * c_b ; u_t = (a_t * zcum) * civ
            for tcc in range(2):
                sl = slice(tcc * 512, (tcc + 1) * 512)
                for h in range(2):
                    nc.gpsimd.tensor_mul(U[0][:, h, sl], A[4][:, h, sl], C[0][:, h, sl])
                    nc.vector.tensor_mul(U[1][:, h, sl], A[3][:, h, sl], C[2][:, h, sl])
                    nc.vector.tensor_mul(U[1][:, h, sl], U[1][:, h, sl], CIV[:, h, sl])

            # ---- final: out[t, d] = sum_r u[r, t] G[r, d], scaled by 1/counts ----
            for tb in range(NTB):
                ot = outp.tile([128, D], f32, name="out_t", tag="out_t")
                tsl = slice(tb * 128, (tb + 1) * 128)
                for nh in range(2):
                    ps = pf.tile([128, 512], f32, name="psF", tag="psF")
                    dsl = slice(nh * 512, (nh + 1) * 512)
                    dsl_t = slice(D + nh * 512, D + (nh + 1) * 512)
                    mm(ps, U[0][:, 0, tsl], G[:, 0, dsl], start=True, stop=False)
                    mm(ps, U[0][:, 1, tsl], G[:, 1, dsl], start=False, stop=False)
                    mm(ps, U[1][:, 0, tsl], G[:, 0, dsl_t], start=False, stop=False)
                    mm(ps, U[1][:, 1, tsl], G[:, 1, dsl_t], start=False, stop=True)
                    nc.scalar.activation(
                        ot[:, dsl], ps, Copy, scale=invp[:, tb:tb + 1]
                    )
                nc.sync.dma_start(out_d.ap()[tsl, :], ot[:, :])
    nc.compile()
    return nc


def get_nc():
    if "nc" not in _CACHE:
        _CACHE["nc"] = _build_nc()
    return _CACHE["nc"]


def make_in_maps(inputs):
    """Host-side fusion + sharding. Returns (in_maps, bias_out)."""
    import ml_dtypes

    bf16 = ml_dtypes.bfloat16
    f = lambda k: np.ascontiguousarray(np.asarray(inputs[k], dtype=np.float32))
    x = f("x")
    Wq, Wk, Wo = f("Wq"), f("Wk"), f("Wo")
    U_b, V_b, W_b = f("U_b"), f("V_b"), f("W_b")
    U_t, V_t, W_t, X_t = f("U_t"), f("V_t"), f("W_t"), f("X_t")
    bias_b, bias_t = f("bias_b"), f("bias_t")
    alpha = float(np.asarray(inputs["alpha"]))

    P_cb = Wk.T @ W_b
    P_ct = Wk.T @ W_t
    P_ab = Wq.T @ V_b
    P_at = Wq.T @ V_t
    # column order: [P_cb | P_ct | X_t | P_at | P_ab]
    P_all = np.concatenate([P_cb, P_ct, X_t, P_at, P_ab], axis=1).astype(bf16)
    Gb = (Wo @ U_b).T
    Gt = alpha * (Wo @ U_t).T
    G_all = round_f32r(np.concatenate([Gb, Gt], axis=1))

    xb = x.astype(bf16)  # device consumes bf16 x
    P_f32 = P_all.astype(np.float32)
    xs = xb.astype(np.float64)[:, :TL, :].sum(axis=1).astype(np.float32)  # [B, D]
    prev_cb = xs @ P_f32[:, 0:R]
    prev_ct = xs @ P_f32[:, R:2 * R]
    prev_z = xs @ P_f32[:, 2 * R:3 * R]

    in_maps = []
    for core in range(NCORES):
        b, h = divmod(core, 2)
        xT = np.ascontiguousarray(xb[b, h * TL:(h + 1) * TL, :].T)
        PX = np.ascontiguousarray(np.concatenate([P_all, xT], axis=1))
        if h == 0:
            prev = np.zeros((128, 6), np.float32)
        else:
            prev = np.ascontiguousarray(
                np.stack(
                    [
                        prev_cb[b, :128], prev_cb[b, 128:],
                        prev_ct[b, :128], prev_ct[b, 128:],
                        prev_z[b, :128], prev_z[b, 128:],
                    ],
                    axis=1,
                )
            )
        counts = np.arange(h * TL + 1, (h + 1) * TL + 1, dtype=np.float64)
        invc = (1.0 / counts).astype(np.float32)
        invp = np.ascontiguousarray(invc.reshape(NTB, 128).T)
        in_maps.append(
            {
                "PX": PX,
                "G_all": G_all,
                "prev": prev,
                "invc_p": invp,
                "invc_row": np.ascontiguousarray(invc),
            }
        )
    bias_out = (bias_b + alpha * bias_t) @ Wo.T
    return in_maps, bias_out


def kernel(**inputs):
    global LAST_RESULTS
    from concourse.bass_utils import run_bass_kernel_spmd

    in_maps, bias_out = make_in_maps(inputs)
    nc = get_nc()
    res = run_bass_kernel_spmd(nc, in_maps, core_ids=list(range(NCORES)))
    LAST_RESULTS = res
    out = np.empty((B, T, D), np.float32)
    for core in range(NCORES):
        b, h = divmod(core, 2)
        out[b, h * TL:(h + 1) * TL, :] = res.results[core]["out"]
    if np.any(bias_out != 0.0):
        out += bias_out[None, None, :]
    return out
